# revision 1
# baseline (speedup 1.0000x reference)
"""Bass/Trainium2 kernel for a 2-layer GCN (GCNConv -> ReLU -> GCNConv ->
ReLU -> global_mean_pool -> Linear), distributed over 8 NeuronCores.

Strategy (graph/data parallel, per the sharding hint): nodes are
partitioned into 8 contiguous ranges of 6250. Each core aggregates the
edges whose *destination* lies in its range, using hardware gather DMAs
(SWDGE dma_gather) of source-node feature rows from the full
feature matrix in DRAM, a selection-matrix matmul for the weighted
segment-sum (accumulated in PSUM), then the dense GEMM X@W + bias +
ReLU.  Layer 1 aggregates the *input* features (aggregate-then-multiply
— valid because aggregation is linear — which halves gather traffic:
256-wide instead of 512-wide rows).  h1 shards are gathered on the host
between the two launches; layer 2 re-uses the identical edge metadata
and additionally accumulates the per-graph pooling partial sums via one
more matmul.  The final (tiny) mean + [512,1] linear runs on host.
"""
import sys
sys.path.insert(0, "/opt/trn_rl_repo")

import numpy as np
import ml_dtypes
from contextlib import ExitStack

from concourse import mybir
import concourse.bacc as bacc
import concourse.tile as tile
from concourse.bass_utils import run_bass_kernel_spmd

P = 128
N_NODES = 50000
N_EDGES = 800000
IN_CH = 256
HID = 512
N_GRAPHS = 64
NCORES = 8
NPC = N_NODES // NCORES            # 6250 nodes per core
NBLK = (NPC + P - 1) // P          # 49 blocks (last one has 106 rows)
LASTV = NPC - (NBLK - 1) * P       # 106
SPLIT = 32768                      # int16 gather-index limit
GCAP = 4                           # max 128-idx chunks per dma_gather instruction

F32 = mybir.dt.float32
BF16 = mybir.dt.bfloat16


def _build_layer(F_in, F_out, L_list, H_list, n_src_total, layer2):
    """Build + compile the bass module for one GCN layer (SPMD, per-core)."""
    C_list = [l + h for l, h in zip(L_list, H_list)]
    CMAX = max(C_list)
    TOTC = sum(C_list)
    KT = F_in // P

    nc = bacc.Bacc("TRN2", target_bir_lowering=False, debug=False)
    xsrc_d = nc.dram_tensor("xsrc", [n_src_total, F_in], BF16, kind="ExternalInput")
    idxs_d = nc.dram_tensor("idxs", [P, 8 * TOTC], mybir.dt.int16, kind="ExternalInput")
    dstloc_d = nc.dram_tensor("dstloc", [P, TOTC], F32, kind="ExternalInput")
    normv_d = nc.dram_tensor("normv", [P, TOTC], F32, kind="ExternalInput")
    iota_d = nc.dram_tensor("iota", [P, CMAX, P], F32, kind="ExternalInput")
    ident_d = nc.dram_tensor("ident", [P, P], BF16, kind="ExternalInput")
    w_d = nc.dram_tensor("w", [P, KT, F_out], BF16, kind="ExternalInput")
    bias_d = nc.dram_tensor("bias", [P, F_out], F32, kind="ExternalInput")
    if layer2:
        batchloc_d = nc.dram_tensor("batchloc", [P, NBLK], F32, kind="ExternalInput")
        pout_d = nc.dram_tensor("pout", [N_GRAPHS, F_out], F32, kind="ExternalOutput")
    else:
        hout_d = nc.dram_tensor("hout", [NPC, F_out], BF16, kind="ExternalOutput")

    with tile.TileContext(nc) as tc, ExitStack() as ctx:
        gat_bufs = 4 if F_in <= 256 else 3
        const = ctx.enter_context(tc.tile_pool(name="const", bufs=1))
        gat = ctx.enter_context(tc.tile_pool(name="gat", bufs=gat_bufs))
        msel = ctx.enter_context(tc.tile_pool(name="msel", bufs=2))
        work = ctx.enter_context(tc.tile_pool(name="work", bufs=3))
        zps = ctx.enter_context(tc.tile_pool(name="zps", bufs=2, space="PSUM"))
        tps = ctx.enter_context(tc.tile_pool(name="tps", bufs=2, space="PSUM"))
        hps = ctx.enter_context(tc.tile_pool(name="hps", bufs=2, space="PSUM"))
        if layer2:
            pps = ctx.enter_context(tc.tile_pool(name="pps", bufs=1, space="PSUM"))

    # constants, loaded once
        idxs_sb = const.tile([P, 8 * TOTC], mybir.dt.int16)
        nc.sync.dma_start(idxs_sb[:], idxs_d[:])
        dstloc_sb = const.tile([P, TOTC], F32)
        nc.sync.dma_start(dstloc_sb[:], dstloc_d[:])
        normv_sb = const.tile([P, TOTC], F32)
        nc.sync.dma_start(normv_sb[:], normv_d[:])
        iota_sb = const.tile([P, CMAX, P], F32)
        nc.sync.dma_start(iota_sb[:], iota_d[:])
        ident_sb = const.tile([P, P], BF16)
        nc.sync.dma_start(ident_sb[:], ident_d[:])
        w_sb = const.tile([P, KT, F_out], BF16)
        nc.sync.dma_start(w_sb[:], w_d[:])
        bias_sb = const.tile([P, F_out], F32)
        nc.sync.dma_start(bias_sb[:], bias_d[:])
        if layer2:
            batchloc_sb = const.tile([P, NBLK], F32)
            nc.sync.dma_start(batchloc_sb[:], batchloc_d[:])
            pool_ps = pps.tile([N_GRAPHS, F_out], F32)

        ioff = 0
        coff = 0
        for b in range(NBLK):
            Lb, Hb = L_list[b], H_list[b]
            Cb = Lb + Hb
            rows = LASTV if b == NBLK - 1 else P

            xg = gat.tile([P, CMAX, F_in], BF16, tag="xg")
            for base, Kb, lo in ((0, Lb, True), (Lb, Hb, False)):
                srcv = xsrc_d[0:SPLIT, :] if lo else xsrc_d[SPLIT:n_src_total, :]
                done = 0
                while done < Kb:
                    g = min(GCAP, Kb - done)
                    nc.gpsimd.dma_gather(
                        xg[:, base + done:base + done + g, :], srcv,
                        idxs_sb[:, ioff:ioff + 8 * g], g * P, g * P, F_in)
                    ioff += 8 * g
                    done += g

            # selection matrix M[e, d] = (dstloc[e]==d) * norm[e]
            M = msel.tile([P, CMAX, P], BF16, tag="M")
            nc.vector.tensor_tensor(
                out=M[:, :Cb, :],
                in0=dstloc_sb[:, coff:coff + Cb].to_broadcast([P, Cb, P]),
                in1=iota_sb[:, :Cb, :],
                op=mybir.AluOpType.is_equal)
            nc.vector.tensor_tensor(
                out=M[:, :Cb, :],
                in0=M[:, :Cb, :],
                in1=normv_sb[:, coff:coff + Cb].to_broadcast([P, Cb, P]),
                op=mybir.AluOpType.mult)

            # weighted segment-sum: z[d, f] += sum_e M[e, d] * xg[e, f]
            z_ps = zps.tile([P, F_in], F32)
            for j in range(Cb):
                nc.tensor.matmul(
                    z_ps[:], M[:, j, :], xg[:, j, :],
                    start=(j == 0), stop=(j == Cb - 1))

            z_sb = work.tile([P, F_in], BF16, tag="z")
            nc.vector.tensor_copy(z_sb[:], z_ps[:])
            zT = work.tile([P, KT, P], BF16, tag="zT")
            for k in range(KT):
                t_ps = tps.tile([P, P], BF16)
                nc.tensor.transpose(t_ps[:], z_sb[:, k * P:(k + 1) * P], ident_sb[:])
                nc.vector.tensor_copy(zT[:, k, :], t_ps[:])

            h_ps = hps.tile([P, F_out], F32)
            for k in range(KT):
                nc.tensor.matmul(
                    h_ps[:], zT[:, k, :], w_sb[:, k, :],
                    start=(k == 0), stop=(k == KT - 1))

            h_sb = work.tile([P, F_out], BF16, tag="h")
            nc.vector.tensor_add(h_sb[:], h_ps[:], bias_sb[:])
            nc.vector.tensor_scalar_max(h_sb[:], h_sb[:], 0.0)

            if layer2:
                G = msel.tile([P, 1, N_GRAPHS], BF16, tag="G")
                nc.vector.tensor_tensor(
                    out=G[:],
                    in0=batchloc_sb[:, b:b + 1].to_broadcast([P, 1, N_GRAPHS]),
                    in1=iota_sb[:, 0:1, :N_GRAPHS],
                    op=mybir.AluOpType.is_equal)
                nc.tensor.matmul(
                    pool_ps[:], G[:, 0, :], h_sb[:],
                    start=(b == 0), stop=(b == NBLK - 1), skip_group_check=True)
            else:
                nc.sync.dma_start(hout_d[b * P:b * P + rows, :], h_sb[:rows, :])
            coff += Cb

        if layer2:
            p_sb = work.tile([N_GRAPHS, F_out], F32, tag="p")
            nc.vector.tensor_copy(p_sb[:], pool_ps[:])
            nc.sync.dma_start(pout_d[:, :], p_sb[:])

    nc.compile()
    return nc


def _preprocess(src, dst, ew, batch):
    """Sort edges by dst, bucket per (core, block), split by the int16
    gather-index boundary, append self-loops, and pack the gather-index /
    selection-metadata streams in the layouts the kernel expects."""
    deg = np.bincount(dst, weights=ew.astype(np.float64), minlength=N_NODES)
    deg = deg.astype(np.float32) + np.float32(1.0)
    dinv = (np.float32(1.0) / np.sqrt(deg)).astype(np.float32)
    norm = (dinv[src] * ew * dinv[dst]).astype(np.float32)

    order = np.argsort(dst, kind="stable")
    ds = dst[order]
    ss = src[order]
    ns = norm[order]

    # per-(core, block) source/dstloc/norm lists incl. self loops
    blk_src = [[None] * NBLK for _ in range(NCORES)]
    blk_dl = [[None] * NBLK for _ in range(NCORES)]
    blk_nv = [[None] * NBLK for _ in range(NCORES)]
    n_lo = np.zeros((NCORES, NBLK), dtype=np.int64)
    n_hi = np.zeros((NCORES, NBLK), dtype=np.int64)

    bnds = []
    for c in range(NCORES):
        for b in range(NBLK):
            bnds.append(c * NPC + b * P)
    bnds.append(N_NODES)
    cuts = np.searchsorted(ds, np.asarray(bnds))

    for c in range(NCORES):
        for b in range(NBLK):
            g0 = c * NPC + b * P
            g1 = min(g0 + P, (c + 1) * NPC)
            i0, i1 = cuts[c * NBLK + b], cuts[c * NBLK + b + 1]
            s_e = ss[i0:i1]
            d_e = (ds[i0:i1] - g0).astype(np.float32)
            n_e = ns[i0:i1]
            s_self = np.arange(g0, g1, dtype=np.int64)
            d_self = np.arange(g1 - g0, dtype=np.float32)
            n_self = (dinv[g0:g1] * dinv[g0:g1]).astype(np.float32)
            s_all = np.concatenate([s_e, s_self])
            d_all = np.concatenate([d_e, d_self])
            n_all = np.concatenate([n_e, n_self])
            lo = s_all < SPLIT
            blk_src[c][b] = (s_all[lo], s_all[~lo] - SPLIT)
            blk_dl[c][b] = (d_all[lo], d_all[~lo])
            blk_nv[c][b] = (n_all[lo], n_all[~lo])
            n_lo[c, b] = int(lo.sum())
            n_hi[c, b] = int((~lo).sum())

    L_list = [int(-(-n_lo[:, b].max() // P)) for b in range(NBLK)]
    H_list = [int(-(-n_hi[:, b].max() // P)) for b in range(NBLK)]
    TOTC = sum(L_list) + sum(H_list)

    idx_cols = np.zeros((NCORES, P, 8 * TOTC), dtype=np.int16)
    dstloc = np.full((NCORES, P, TOTC), -5.0, dtype=np.float32)
    normv = np.zeros((NCORES, P, TOTC), dtype=np.float32)
    batchloc = np.full((NCORES, P, NBLK), -5.0, dtype=np.float32)

    for c in range(NCORES):
        ioff = 0
        coff = 0
        for b in range(NBLK):
            Lb, Hb = L_list[b], H_list[b]
            for part, Kb in ((0, Lb), (1, Hb)):
                s_p = blk_src[c][b][part]
                pad = np.zeros(Kb * P, dtype=np.int16)
                pad[:len(s_p)] = s_p
                idx_cols[c, :, ioff:ioff + 8 * Kb] = np.tile(
                    pad.reshape(Kb * 8, 16).T, (8, 1))
                ioff += 8 * Kb
                dl = np.full(Kb * P, -5.0, dtype=np.float32)
                dl[:len(s_p)] = blk_dl[c][b][part]
                nv = np.zeros(Kb * P, dtype=np.float32)
                nv[:len(s_p)] = blk_nv[c][b][part]
                dstloc[c, :, coff:coff + Kb] = dl.reshape(Kb, P).T
                normv[c, :, coff:coff + Kb] = nv.reshape(Kb, P).T
                coff += Kb
            g0 = c * NPC + b * P
            g1 = min(g0 + P, (c + 1) * NPC)
            batchloc[c, :g1 - g0, b] = batch[g0:g1]

    return dict(L_list=L_list, H_list=H_list, idx_cols=idx_cols,
                dstloc=dstloc, normv=normv, batchloc=batchloc, dinv=dinv)


def _const_inputs(CMAX):
    iota = np.tile(np.arange(P, dtype=np.float32), (P, 1))
    iota3 = np.ascontiguousarray(np.broadcast_to(iota[:, None, :], (P, CMAX, P)))
    ident = np.eye(P, dtype=ml_dtypes.bfloat16)
    return iota3, ident


def _w_arrange(W):
    F_in, F_out = W.shape
    KT = F_in // P
    return np.ascontiguousarray(
        W.reshape(KT, P, F_out).transpose(1, 0, 2)).astype(ml_dtypes.bfloat16)


def _run_gcn(x, edge_index, edge_weight, batch, W1, b1, W2, b2, Wl, bl,
             trace=False):
    """Returns (out [64,1] fp32, exec_time_ns or None)."""
    src = np.asarray(edge_index[0]).astype(np.int64)
    dst = np.asarray(edge_index[1]).astype(np.int64)
    ew = np.asarray(edge_weight).astype(np.float32)
    batch = np.asarray(batch).astype(np.int64)
    x = np.ascontiguousarray(np.asarray(x, dtype=np.float32))

    pre = _preprocess(src, dst, ew, batch)
    L_list, H_list = pre["L_list"], pre["H_list"]
    CMAX = max(l + h for l, h in zip(L_list, H_list))
    iota3, ident = _const_inputs(CMAX)

    nc1 = _build_layer(IN_CH, HID, L_list, H_list, N_NODES, layer2=False)
    nc2 = _build_layer(HID, HID, L_list, H_list, N_NODES, layer2=True)

    bias1 = np.tile(np.asarray(b1, dtype=np.float32)[None, :], (P, 1))
    bias2 = np.tile(np.asarray(b2, dtype=np.float32)[None, :], (P, 1))
    w1 = _w_arrange(np.asarray(W1, dtype=np.float32))
    w2 = _w_arrange(np.asarray(W2, dtype=np.float32))

    x_bf = x.astype(ml_dtypes.bfloat16)
    in_maps1 = []
    for c in range(NCORES):
        in_maps1.append({
            "xsrc": x_bf,
            "idxs": pre["idx_cols"][c],
            "dstloc": pre["dstloc"][c],
            "normv": pre["normv"][c],
            "iota": iota3,
            "ident": ident,
            "w": w1,
            "bias": bias1,
        })
    r1 = run_bass_kernel_spmd(nc1, in_maps1, core_ids=list(range(NCORES)),
                              trace=trace)
    h1 = np.concatenate([r1.results[c]["hout"] for c in range(NCORES)], axis=0)

    in_maps2 = []
    for c in range(NCORES):
        in_maps2.append({
            "xsrc": h1,
            "idxs": pre["idx_cols"][c],
            "dstloc": pre["dstloc"][c],
            "normv": pre["normv"][c],
            "iota": iota3,
            "ident": ident,
            "w": w2,
            "bias": bias2,
            "batchloc": pre["batchloc"][c],
        })
    r2 = run_bass_kernel_spmd(nc2, in_maps2, core_ids=list(range(NCORES)),
                              trace=trace)
    pool = np.sum([r2.results[c]["pout"] for c in range(NCORES)], axis=0)

    cnt = np.bincount(batch, minlength=N_GRAPHS).astype(np.float32)
    g = pool / np.maximum(cnt, 1.0)[:, None]
    out = (g.astype(np.float32) @ np.asarray(Wl, dtype=np.float32)
           + np.asarray(bl, dtype=np.float32))

    exec_ns = None
    if trace:
        t1 = getattr(r1, "exec_time_ns", None)
        t2 = getattr(r2, "exec_time_ns", None)
        if t1 is not None and t2 is not None:
            exec_ns = t1 + t2
    return out.astype(np.float32), exec_ns


def kernel(**inputs):
    out, _ = _run_gcn(
        inputs["x"], inputs["edge_index"], inputs["edge_weight"],
        inputs["batch"], inputs["W1"], inputs["b1"], inputs["W2"],
        inputs["b2"], inputs["Wl"], inputs["bl"])
    return out


def _exec_layer(nc, in_maps, bench_iters=0):
    """Execute a compiled layer on the 8 cores via PJRT (same lowering as
    run_bass_kernel_spmd under axon), optionally re-running it
    `bench_iters` times with device-resident inputs to wall-clock the
    execution.  Returns (per-core results list, best_exec_seconds|None)."""
    import time
    import jax
    from jax.experimental.shard_map import shard_map
    from jax.sharding import Mesh, PartitionSpec, NamedSharding
    from concourse import bass2jax, mybir as mb

    bass2jax.install_neuronx_cc_hook()
    n_cores = len(in_maps)
    partition_name = (nc.partition_id_tensor.name if nc.partition_id_tensor
                      else None)
    in_names, out_names, out_avals, zero_outs = [], [], [], []
    for alloc in nc.m.functions[0].allocations:
        if not isinstance(alloc, mb.MemoryLocationSet):
            continue
        name = alloc.memorylocations[0].name
        if alloc.kind == "ExternalInput":
            if name != partition_name:
                in_names.append(name)
        elif alloc.kind == "ExternalOutput":
            out_names.append(name)
            shape = tuple(alloc.tensor_shape)
            dtype = mb.dt.np(alloc.dtype)
            out_avals.append(jax.core.ShapedArray(shape, dtype))
            zero_outs.append(np.zeros(shape, dtype))
    n_params = len(in_names)
    n_outs = len(out_avals)
    all_in_names = list(in_names) + out_names
    if partition_name is not None:
        all_in_names.append(partition_name)

    def _body(*args):
        operands = list(args)
        if partition_name is not None:
            operands.append(bass2jax.partition_id_tensor())
        outs = bass2jax._bass_exec_p.bind(
            *operands,
            out_avals=tuple(out_avals),
            in_names=tuple(all_in_names),
            out_names=tuple(out_names),
            lowering_input_output_aliases=(),
            sim_require_finite=True,
            sim_require_nnan=True,
            nc=nc,
        )
        return tuple(outs)

    devices = jax.devices()[:n_cores]
    mesh = Mesh(np.asarray(devices), ("core",))
    spec = PartitionSpec("core")
    in_specs = (spec,) * (n_params + n_outs)
    out_specs = (spec,) * n_outs
    donate = tuple(range(n_params, n_params + n_outs))
    sharded = jax.jit(
        shard_map(_body, mesh=mesh, in_specs=in_specs, out_specs=out_specs,
                  check_rep=False),
        donate_argnums=donate, keep_unused=True)

    sh = NamedSharding(mesh, spec)
    concat_in = [
        jax.device_put(
            np.concatenate([np.asarray(in_maps[c][nm]) for c in range(n_cores)],
                           axis=0), sh)
        for nm in in_names]
    def put_zeros():
        return [jax.device_put(
                    np.zeros((n_cores * z.shape[0], *z.shape[1:]), z.dtype), sh)
                for z in zero_outs]

    out_arrs = sharded(*concat_in, *put_zeros())
    jax.block_until_ready(out_arrs)
    results = [
        {nm: np.asarray(out_arrs[i]).reshape(n_cores, *out_avals[i].shape)[c]
         for i, nm in enumerate(out_names)}
        for c in range(n_cores)]

    best = None
    for _ in range(bench_iters):
        zs = put_zeros()
        jax.block_until_ready(zs)
        t0 = time.perf_counter()
        o = sharded(*concat_in, *zs)
        jax.block_until_ready(o)
        dt = time.perf_counter() - t0
        best = dt if best is None or dt < best else best
    return results, best

